# revision 5
# baseline (speedup 1.0000x reference)
"""Distributed Trainium2 kernel for a causal multi-head self-attention block.

  out = softmax_causal((x@Wq+bq)(x@Wk+bk)^T / sqrt(Dh)) (x@Wv+bv) @ W_out + b_out

Sharding (8 NeuronCores, tensor-parallel over heads):
  - Each core owns 2 of the 16 heads, both batches -> 4 (batch, head) units.
  - Host packs x / weights partition-contiguous so every DMA reads >=8KB
    linear per partition (full-rate HBM streams).
  - Per batch, QKV s-chunks and attention s-chunks are interleaved so the
    scalar engine's exp work (the attention bottleneck) overlaps the QKV
    matmul stream, and the per-batch AllToAll fires as early as possible.
  - Attention: scoresT = kT-tile.T @ qT (t on partitions, s free); exp with
    no max subtraction (scores ~ N(0,1), safe); causal at tile granularity
    with a triangular mask multiply on diagonal tiles; PV matmul uses
    [v | ones] so the softmax denominator falls out of PSUM column 128.
    Chunk tails (normalize + PE-transpose + bias) are deferred one chunk so
    the PE never waits on the vector engine.
  - Two AllToAlls per batch (one per owned head) redistribute head-shards ->
    token-shards; b1's collectives overlap the b0 output projection.
  - Output projection is token-parallel with the full W_out; host
    reassembles, transposes, and adds b_out.

All matmul operands are bf16 (1 cycle/row on the PE), accumulation f32.
"""

import math
import numpy as np
import ml_dtypes

import sys

for _p in ("/opt/trn_rl_repo",):
    if _p not in sys.path:
        sys.path.insert(0, _p)

import concourse.bass as bass
import concourse.bacc as bacc
import concourse.mybir as mybir
import concourse.tile as tile
from concourse.bass_utils import run_bass_kernel_spmd

BF16 = mybir.dt.bfloat16
F32 = mybir.dt.float32
NPBF16 = ml_dtypes.bfloat16

B, S, D = 2, 2048, 2048
H, DH = 16, 128
NC = 8
HL = H // NC            # heads per core = 2
SC = 512                # s-chunk (free dim of scores matmul)
NCH = S // SC           # 4 s-chunks per batch
NT = S // 128           # 16 t-tiles per batch
TOKB = S // NC          # 256 tokens owned per core per batch
INV_SQRT_DH = 1.0 / math.sqrt(DH)


def build_kernel():
    nc = bacc.Bacc("TRN2", target_bir_lowering=False, debug=False, num_devices=NC)

    # host-packed, partition-contiguous layouts
    xp = nc.declare_dram_parameter("xp", [B, NCH, 128, 16, SC], BF16, isOutput=False)
    wq = nc.declare_dram_parameter("wq", [128, HL, 16, 128], BF16, isOutput=False)
    wk = nc.declare_dram_parameter("wk", [128, HL, 16, 128], BF16, isOutput=False)
    wv = nc.declare_dram_parameter("wv", [128, 16, HL * 128], BF16, isOutput=False)
    bq = nc.declare_dram_parameter("bq", [128, HL, 1], F32, isOutput=False)
    bk = nc.declare_dram_parameter("bk", [128, HL, 1], F32, isOutput=False)
    bv = nc.declare_dram_parameter("bv", [128, HL, 1], F32, isOutput=False)
    w_out = nc.declare_dram_parameter("w_out", [128, 16, D], BF16, isOutput=False)
    ident = nc.declare_dram_parameter("ident", [128, 128], BF16, isOutput=False)
    maskp = nc.declare_dram_parameter("maskp", [128, 128], BF16, isOutput=False)
    out = nc.declare_dram_parameter("out", [B, D, TOKB], F32, isOutput=True)

    with tile.TileContext(nc) as tc:
        with (
            tc.tile_pool(name="wpool", bufs=1) as wpool,
            tc.tile_pool(name="xpool", bufs=4) as xpool,
            tc.tile_pool(name="qkv", bufs=1) as qkvpool,
            tc.tile_pool(name="expp", bufs=4) as expp,
            tc.tile_pool(name="small", bufs=4) as small,
            tc.tile_pool(name="wo", bufs=1) as wopool,
            tc.tile_pool(name="rcv", bufs=2) as rcvpool,
            tc.tile_pool(name="outp", bufs=2) as outp,
            tc.tile_pool(name="psum", bufs=2, space="PSUM") as psum,
            tc.tile_pool(name="dram", bufs=1, space="DRAM") as dram,
        ):
            # ---- weights first on each HW-DGE ring (startup-critical) ----
            wq_t = wpool.tile([128, HL, 16, 128], BF16, tag="wq")
            wk_t = wpool.tile([128, HL, 16, 128], BF16, tag="wk")
            wv_t = wpool.tile([128, 16, HL * 128], BF16, tag="wv")
            bq_t = wpool.tile([128, HL, 1], F32, tag="bq")
            bk_t = wpool.tile([128, HL, 1], F32, tag="bk")
            bv_t = wpool.tile([128, HL, 1], F32, tag="bv")
            id_t = wpool.tile([128, 128], BF16, tag="ident")
            mask_t = wpool.tile([128, 128], BF16, tag="maskp")
            nc.sync.dma_start(wq_t[:], wq[:])
            nc.scalar.dma_start(wk_t[:], wk[:])
            nc.scalar.dma_start(wv_t[:], wv[:])
            nc.gpsimd.dma_start(bq_t[:], bq[:])
            nc.gpsimd.dma_start(bk_t[:], bk[:])
            nc.gpsimd.dma_start(bv_t[:], bv[:])
            nc.gpsimd.dma_start(id_t[:], ident[:])
            nc.gpsimd.dma_start(mask_t[:], maskp[:])

            # per-(batch, head) A2A bounce buffers (DRAM)
            a2a_in = [
                [
                    dram.tile([NC, 2, 128, 128], BF16, tag=f"a2a_in{b}_{hl}",
                              name=f"a2a_in{b}_{hl}")
                    for hl in range(HL)
                ]
                for b in range(B)
            ]
            a2a_out = [
                [
                    dram.tile([NC, 2, 128, 128], BF16, tag=f"a2a_out{b}_{hl}",
                              name=f"a2a_out{b}_{hl}")
                    for hl in range(HL)
                ]
                for b in range(B)
            ]

            def load_x_chunk(b, c):
                xt = xpool.tile([128, 16, SC], BF16, tag="xt",
                                name=f"xt_{b}_{c}")
                nc.sync.dma_start(xt[:, 0:8], xp[b, c, :, 0:8])
                nc.scalar.dma_start(xt[:, 8:16], xp[b, c, :, 8:16])
                return xt

            def qkv_chunk(b, scn, xt, qTb, kTb, vvb):
                for hl in range(HL):
                    for w_t, b_t, dst in ((wq_t, bq_t, qTb), (wk_t, bk_t, kTb)):
                        ps = psum.tile([128, SC], F32, tag="mm",
                                       name=f"psqk_{b}_{scn}_{hl}_{id(dst)}")
                        for d in range(16):
                            nc.tensor.matmul(
                                ps[:], w_t[:, hl, d], xt[:, d],
                                start=(d == 0), stop=(d == 15),
                            )
                        nc.vector.tensor_scalar_add(
                            dst[:, hl, scn * SC : (scn + 1) * SC],
                            ps[:], b_t[:, hl],
                        )
                for ts in range(SC // 128):
                    ps = psum.tile([128, HL * 128], F32, tag="mm",
                                   name=f"psv_{b}_{scn}_{ts}")
                    for d in range(16):
                        nc.tensor.matmul(
                            ps[:],
                            xt[:, d, ts * 128 : (ts + 1) * 128],
                            wv_t[:, d],
                            start=(d == 0), stop=(d == 15),
                        )
                    tt_idx = scn * (SC // 128) + ts
                    for hl in range(HL):
                        nc.vector.tensor_copy(
                            vvb[:, hl, tt_idx, 0:128],
                            ps[:, hl * 128 : (hl + 1) * 128],
                        )

            def attn_chunk(b, hl, scn, qTb, kTb, vvb):
                """Scores + exp + PV for chunk scn of unit (b, hl).
                Returns the deferred tail closure."""
                o2 = [
                    psum.tile([128, 2, 129], F32, tag="o2", bufs=4,
                              name=f"o2_{b}_{hl}_{scn}_{i}")
                    for i in range(2)
                ]
                for tt in range(4 * scn + 4):
                    off = max(0, tt - 4 * scn)  # first live s-subtile
                    nlive = 4 - off
                    s0 = scn * SC + off * 128
                    sp = psum.tile([128, SC], F32, tag="mm",
                                   name=f"sp_{b}_{hl}_{scn}_{tt}")
                    nc.tensor.matmul(
                        sp[:, : nlive * 128],
                        kTb[:, hl, tt * 128 : (tt + 1) * 128],
                        qTb[:, hl, s0 : (scn + 1) * SC],
                        start=True, stop=True,
                    )
                    ex = expp.tile([128, SC], BF16, tag="ex",
                                   name=f"ex_{b}_{hl}_{scn}_{tt}")
                    nc.scalar.activation(
                        ex[:, : nlive * 128], sp[:, : nlive * 128],
                        mybir.ActivationFunctionType.Exp,
                        scale=INV_SQRT_DH,
                    )
                    if tt >= 4 * scn:  # diagonal sub-block: causal mask
                        nc.vector.tensor_mul(
                            ex[:, 0:128], ex[:, 0:128], mask_t[:]
                        )
                    for ss in range(off, 4):
                        st = 4 * scn + ss
                        # start=True clears has_written BANK-wide; only
                        # the first matmul touching each o2 bank may set
                        # it. The sibling slice's first write relies on
                        # the cleared has_written bits (overwrite mode).
                        nc.tensor.matmul(
                            o2[ss // 2][:, ss % 2, :],
                            ex[:, (ss - off) * 128 : (ss - off + 1) * 128],
                            vvb[:, hl, tt],
                            start=(tt == 0 and ss % 2 == 0),
                            stop=(tt == st),
                        )

                def tail():
                    for ss in range(4):
                        st = 4 * scn + ss
                        o2t = o2[ss // 2]
                        rc = small.tile([128, 1], F32, tag="rc",
                                        name=f"rc_{b}_{hl}_{scn}_{ss}")
                        nc.vector.reciprocal(rc[:], o2t[:, ss % 2, 128:129])
                        an = small.tile([128, 128], BF16, tag="an",
                                        name=f"an_{b}_{hl}_{scn}_{ss}")
                        nc.vector.tensor_scalar_mul(
                            an[:], o2t[:, ss % 2, 0:128], rc[:]
                        )
                        tp = psum.tile([128, 128], BF16, tag="tp", bufs=2,
                                       name=f"tp_{b}_{hl}_{scn}_{ss}")
                        nc.tensor.transpose(tp[:], an[:], id_t[:])
                        at = small.tile([128, 128], BF16, tag="at",
                                        name=f"at_{b}_{hl}_{scn}_{ss}")
                        nc.vector.tensor_scalar_add(at[:], tp[:], bv_t[:, hl])
                        nc.gpsimd.dma_start(
                            a2a_in[b][hl][st // 2, st % 2], at[:]
                        )

                return tail

            def load_rcv(b, rcv):
                for dt in range(16):
                    srcc, shl = dt // HL, dt % HL
                    nc.scalar.dma_start(
                        rcv[:, dt],
                        a2a_out[b][shl][srcc].rearrange("s p m -> p s m"),
                    )

            def proj_phase(b, rcv):
                for oc in range(16):
                    ps = psum.tile([128, TOKB], F32, tag="mm",
                                   name=f"pso_{b}_{oc}")
                    for dt in range(16):
                        nc.tensor.matmul(
                            ps[:],
                            wo_t[:, dt, oc * 128 : (oc + 1) * 128],
                            rcv[:, dt],
                            start=(dt == 0), stop=(dt == 15),
                        )
                    ot = outp.tile([128, TOKB], F32, tag="ot",
                                   name=f"ot_{b}_{oc}")
                    nc.vector.tensor_copy(ot[:], ps[:])
                    nc.sync.dma_start(out[b, oc * 128 : (oc + 1) * 128, :], ot[:])

            # ---------------- program order ----------------
            # b0 x chunks: both HW-DGE rings, right behind the weights.
            xts0 = [load_x_chunk(0, c) for c in range(NCH)]
            # W_out sits behind the b0 x stream on both rings: it only
            # starts once those descriptors drain (~20us), so it cannot
            # starve the startup-critical loads.
            wo_t = wopool.tile([128, 16, D], BF16, tag="wo")
            nc.sync.dma_start(wo_t[:, 0:8], w_out[:, 0:8])
            nc.scalar.dma_start(wo_t[:, 8:16], w_out[:, 8:16])

            rcv_t = [None, None]
            for b in range(B):
                qTb = qkvpool.tile([128, HL, S], BF16, tag="qT", name=f"qT{b}")
                kTb = qkvpool.tile([128, HL, S], BF16, tag="kT", name=f"kT{b}")
                vvb = qkvpool.tile([128, HL, NT, 129], BF16, tag="vv",
                                   name=f"vv{b}")
                nc.gpsimd.memset(vvb[:, :, :, 128:129], 1.0)
                xts = xts0 if b == 0 else xts1
                pending = []
                for scn in range(NCH):
                    qkv_chunk(b, scn, xts[scn], qTb, kTb, vvb)
                    if b == 0:
                        # b1 chunk loads woven in: each slot's WAR dep (on
                        # qkv reads of the b0 chunk it replaces) is correct
                        # at this emission point, and the ring position puts
                        # them behind w_out.
                        if scn == 0:
                            xts1 = []
                        xts1.append(load_x_chunk(1, scn))
                    # tails from chunk scn-1 had the whole qkv chunk above
                    # for their DVE normalize, so the PE transposes in them
                    # never wait on the vector engine.
                    for tl in pending:
                        tl()
                    pending = [
                        attn_chunk(b, hl, scn, qTb, kTb, vvb)
                        for hl in range(HL)
                    ]
                for tl in pending:
                    tl()
                for hl in range(HL):
                    nc.gpsimd.collective_compute(
                        "AllToAll",
                        mybir.AluOpType.bypass,
                        ins=[a2a_in[b][hl].opt()],
                        outs=[a2a_out[b][hl].opt()],
                        replica_groups=[list(range(NC))],
                    )
                rcv_t[b] = rcvpool.tile([128, 16, TOKB], BF16, tag="rcv",
                                        name=f"rcv{b}")
                load_rcv(b, rcv_t[b])

            # proj(0) fills the PE while b1's collectives fly.
            proj_phase(0, rcv_t[0])
            proj_phase(1, rcv_t[1])

    nc.compile()
    return nc


def make_in_maps(x, W_in, b_in, W_out, b_out):
    # x packed so each SBUF partition reads 16KB contiguous per chunk:
    # xp[b, c, p, d, s] = x[b, c*SC+s, d*128+p]
    xp = np.ascontiguousarray(
        x.reshape(B, NCH, SC, 16, 128).transpose(0, 1, 4, 3, 2)
    ).astype(NPBF16)
    ident = np.eye(128, dtype=NPBF16)
    maskp = np.triu(np.ones((128, 128), dtype=np.float32)).astype(NPBF16)
    # w_out[p, d, m] = W_out[d*128+p, m] (32KB contiguous per partition)
    w_out_t = np.ascontiguousarray(
        W_out.reshape(16, 128, D).transpose(1, 0, 2)
    ).astype(NPBF16)

    in_maps = []
    for c in range(NC):
        hs = [2 * c + hl for hl in range(HL)]  # global head ids
        # wq[p, hl, d, m] = W_in[d*128+p, h*128+m]
        wq_c = np.ascontiguousarray(
            np.stack(
                [W_in[:, h * 128 : (h + 1) * 128].reshape(16, 128, 128) for h in hs]
            ).transpose(2, 0, 1, 3)
        ).astype(NPBF16)
        wk_c = np.ascontiguousarray(
            np.stack(
                [
                    W_in[:, D + h * 128 : D + (h + 1) * 128].reshape(16, 128, 128)
                    for h in hs
                ]
            ).transpose(2, 0, 1, 3)
        ).astype(NPBF16)
        # wv[p, d, m] over the HL*128 concatenated head columns
        wv_c = np.ascontiguousarray(
            np.concatenate(
                [
                    W_in[:, 2 * D + h * 128 : 2 * D + (h + 1) * 128].reshape(
                        16, 128, 128
                    )
                    for h in hs
                ],
                axis=2,
            ).transpose(1, 0, 2)
        ).astype(NPBF16)
        bq_c = np.ascontiguousarray(
            np.stack([b_in[h * 128 : (h + 1) * 128] for h in hs], axis=1)
        ).reshape(128, HL, 1).astype(np.float32)
        bk_c = np.ascontiguousarray(
            np.stack([b_in[D + h * 128 : D + (h + 1) * 128] for h in hs], axis=1)
        ).reshape(128, HL, 1).astype(np.float32)
        bv_c = np.ascontiguousarray(
            np.stack([b_in[2 * D + h * 128 : 2 * D + (h + 1) * 128] for h in hs],
                     axis=1)
        ).reshape(128, HL, 1).astype(np.float32)
        in_maps.append(
            {
                "xp": xp,
                "wq": wq_c,
                "wk": wk_c,
                "wv": wv_c,
                "bq": bq_c,
                "bk": bk_c,
                "bv": bv_c,
                "w_out": w_out_t,
                "ident": ident,
                "maskp": maskp,
            }
        )
    return in_maps


_NC_CACHE = {}


def _get_nc():
    if "nc" not in _NC_CACHE:
        _NC_CACHE["nc"] = build_kernel()
    return _NC_CACHE["nc"]


def kernel(x, W_in, b_in, W_out, b_out, _trace=False, **kw):
    x = np.asarray(x, dtype=np.float32)
    W_in = np.asarray(W_in, dtype=np.float32)
    b_in = np.asarray(b_in, dtype=np.float32)
    W_out = np.asarray(W_out, dtype=np.float32)
    b_out = np.asarray(b_out, dtype=np.float32)

    nc = _get_nc()
    in_maps = make_in_maps(x, W_in, b_in, W_out, b_out)
    res = run_bass_kernel_spmd(nc, in_maps, core_ids=list(range(NC)), trace=_trace)
    outf = np.empty((B, S, D), dtype=np.float32)
    for c in range(NC):
        o = np.asarray(res.results[c]["out"])  # [B, D, TOKB]
        for b in range(B):
            outf[b, c * TOKB : (c + 1) * TOKB, :] = o[b].T
    outf += b_out[None, None, :]
    if _trace:
        return outf, res
    return outf
